# revision 2
# baseline (speedup 1.0000x reference)
"""Multi-head attention (B=2, S=2048, E=1024, H=16, causal) on 8 TRN2 NeuronCores.

Sharding: data-parallel over batch (2) x tensor-parallel over head groups (4):
core c handles batch b = c//4 and heads 4*(c%4) .. 4*(c%4)+3.

Per-core device kernel (all matmuls bf16, f32 accumulation):
  phase 1: q^T, k^T = (Wq_g @ Q_b^T + bq_g), ...   layout [d, t]   (d on partitions)
           v       = V_b @ Wv_g^T + bv_g           layout [t, d]   (keys on partitions)
  phase 2: per head: scores^T = k^T . q^T (contract d), exp (no max-subtract;
           scores are O(1) so exp is safe), causal mask by skipping/zeroing
           tiles; attn^T[d, q] = sum_k v_aug[k, d] probs^T[k, q] where v_aug
           carries a ones column that yields the softmax denominator for free.
  phase 3: y_partial[t, e] = attn^T . Wo_g^T   (contract over this core's 256
           head-dims), DMA'd out as f32.
Host side: shard/transpose/cast inputs, then sum the 4 per-core partials of
each batch and add bo.
"""

import math
import os
import sys
from contextlib import ExitStack

for _p in ("/opt/trn_rl_repo", "/opt/pypackages"):
    if _p not in sys.path:
        sys.path.insert(0, _p)

import numpy as np
import ml_dtypes

BF16 = ml_dtypes.bfloat16

B, S, E, H = 2, 2048, 1024, 16
D = E // H                      # 64
N_CORES = 8
GROUPS = N_CORES // B           # 4 head-groups per batch
HPC = H // GROUPS               # 4 heads per core
HD = HPC * D                    # 256 head-dims per core
SCALE = 1.0 / math.sqrt(D)

_BUILD_CACHE = {}


def build_nc(seq_len=S, causal=True, use_mask=False):
    """Build (and bacc-compile) the per-core Bass program. Returns nc."""
    key = (seq_len, causal, use_mask)
    if key in _BUILD_CACHE:
        return _BUILD_CACHE[key]

    import concourse.bass as bass
    import concourse.tile as tile
    import concourse.mybir as mybir
    from concourse import bacc
    from concourse.bass import ts, ds

    f32 = mybir.dt.float32
    bf16 = mybir.dt.bfloat16
    EXP = mybir.ActivationFunctionType.Exp

    SQ = seq_len
    n_tt = SQ // 128            # token tiles (keys / queries / rows)
    n_ch = SQ // 512            # 512-wide query chunks
    n_et = E // 128             # contraction tiles over E

    nc = bacc.Bacc("TRN2", target_bir_lowering=False, debug=False,
                   num_devices=N_CORES)

    QT = nc.dram_tensor("qt_in", [E, SQ], bf16, kind="ExternalInput").ap()
    KT = nc.dram_tensor("kt_in", [E, SQ], bf16, kind="ExternalInput").ap()
    VT = nc.dram_tensor("vt_in", [E, SQ], bf16, kind="ExternalInput").ap()
    WQT = nc.dram_tensor("wqt", [E, HD], bf16, kind="ExternalInput").ap()
    WKT = nc.dram_tensor("wkt", [E, HD], bf16, kind="ExternalInput").ap()
    WVT = nc.dram_tensor("wvt", [E, HD], bf16, kind="ExternalInput").ap()
    WOT = nc.dram_tensor("wot", [HD, E], bf16, kind="ExternalInput").ap()
    BQ = nc.dram_tensor("bq_in", [HD, 1], f32, kind="ExternalInput").ap()
    BK = nc.dram_tensor("bk_in", [HD, 1], f32, kind="ExternalInput").ap()
    BV = nc.dram_tensor("bv_in", [1, HD], f32, kind="ExternalInput").ap()
    TRI = nc.dram_tensor("tri", [128, 128], bf16, kind="ExternalInput").ap()
    if use_mask:
        MSK = nc.dram_tensor("mskt", [SQ, SQ], bf16, kind="ExternalInput").ap()
    Y = nc.dram_tensor("y", [SQ, E], f32, kind="ExternalOutput").ap()

    with tile.TileContext(nc) as tc, ExitStack() as ctx:
        const = ctx.enter_context(tc.tile_pool(name="const", bufs=1))
        stage = ctx.enter_context(tc.tile_pool(name="stage", bufs=1))
        probs_pool = ctx.enter_context(tc.tile_pool(name="probsp", bufs=1))
        work = ctx.enter_context(tc.tile_pool(name="work", bufs=4))
        pp = ctx.enter_context(tc.tile_pool(name="pp", bufs=1, space="PSUM"))

        # ---- constants ------------------------------------------------
        wq_sb = const.tile([128, n_et, HD], bf16, tag="wq", name="wq_sb")
        nc.sync.dma_start(out=wq_sb, in_=WQT.rearrange("(t p) d -> p t d", p=128))
        wk_sb = const.tile([128, n_et, HD], bf16, tag="wk", name="wk_sb")
        nc.sync.dma_start(out=wk_sb, in_=WKT.rearrange("(t p) d -> p t d", p=128))
        wv_sb = const.tile([128, n_et, HD], bf16, tag="wv", name="wv_sb")
        nc.sync.dma_start(out=wv_sb, in_=WVT.rearrange("(t p) d -> p t d", p=128))

        wo_sb = []
        for m in range(HD // 128):
            t_ = const.tile([128, E], bf16, tag=f"wo{m}", name=f"wo_sb{m}")
            nc.sync.dma_start(out=t_, in_=WOT[ts(m, 128), :])
            wo_sb.append(t_)

        bq_sb = const.tile([128, HD // 128], f32, tag="bq", name="bq_sb")
        nc.sync.dma_start(out=bq_sb, in_=BQ.rearrange("(m p) o -> p (m o)", p=128))
        bk_sb = const.tile([128, HD // 128], f32, tag="bk", name="bk_sb")
        nc.sync.dma_start(out=bk_sb, in_=BK.rearrange("(m p) o -> p (m o)", p=128))
        bv_sb = const.tile([128, HD], f32, tag="bv", name="bv_sb")
        nc.gpsimd.dma_start(out=bv_sb, in_=BV.to_broadcast((128, HD)))
        tri_sb = const.tile([128, 128], bf16, tag="tri", name="tri_sb")
        nc.sync.dma_start(out=tri_sb, in_=TRI)

        # ---- staged inputs (full) -------------------------------------
        def load_T(src, prefix):
            tiles = []
            srcr = src.rearrange("(t p) s -> t p s", p=128)
            for i in range(n_et):
                t_ = stage.tile([128, SQ], bf16, tag=f"{prefix}{i}",
                                name=f"{prefix}{i}_sb")
                nc.sync.dma_start(out=t_, in_=srcr[i])
                tiles.append(t_)
            return tiles

        qt_in = load_T(QT, "qti")
        kt_in = load_T(KT, "kti")
        vt_in = load_T(VT, "vti")

        # ---- persistent activations ----------------------------------
        qt_sb = [const.tile([128, SQ], bf16, tag=f"qt{m}", name=f"qt_sb{m}")
                 for m in range(HD // 128)]
        kt_sb = [const.tile([128, SQ], bf16, tag=f"kt{m}", name=f"kt_sb{m}")
                 for m in range(HD // 128)]
        v_sb = const.tile([128, n_tt, HPC, D + 1], bf16, tag="v", name="v_sb")
        nc.vector.memset(v_sb[:, :, :, D:D + 1], 1.0)
        at_sb = [const.tile([128, SQ], bf16, tag=f"at{m}", name=f"at_sb{m}")
                 for m in range(HD // 128)]

        # ---- phase 1a: q^T, k^T  [d, t] -------------------------------
        for x_in, w_sb, b_sb, dst in ((qt_in, wq_sb, bq_sb, qt_sb),
                                      (kt_in, wk_sb, bk_sb, kt_sb)):
            for m in range(HD // 128):
                for nch in range(n_ch):
                    ps = pp.tile([128, 512], f32, tag="proj", bufs=2,
                                 name="proj_ps")
                    for et in range(n_et):
                        nc.tensor.matmul(ps,
                                         w_sb[:, et, ts(m, 128)],
                                         x_in[et][:, ts(nch, 512)],
                                         start=(et == 0), stop=(et == n_et - 1))
                    nc.vector.tensor_scalar_add(dst[m][:, ts(nch, 512)], ps,
                                                b_sb[:, m:m + 1])

        # ---- phase 1b: v  [t, d] --------------------------------------
        for tt in range(n_tt):
            ps = pp.tile([128, HD], f32, tag="proj", bufs=2, name="vproj_ps")
            for et in range(n_et):
                nc.tensor.matmul(ps,
                                 vt_in[et][:, ts(tt, 128)],
                                 wv_sb[:, et, :],
                                 start=(et == 0), stop=(et == n_et - 1))
            nc.vector.tensor_add(v_sb[:, tt, :, 0:D],
                                 ps.rearrange("p (h d) -> p h d", h=HPC),
                                 bv_sb.rearrange("p (h d) -> p h d", h=HPC))

        # ---- phase 2: attention per head-pair -------------------------
        for pr_i in range(HD // 128):           # head-pair index (2 heads)
            for c in range(n_ch):
                nj = min(4 * c + 4, n_tt) if causal else n_tt
                probs = {}
                msk_tiles = {}
                for j in range(nj):
                    diag = causal and (j // 4 == c)
                    q0 = (j - 4 * c) * 128 if diag else 0
                    w = 512 - q0
                    if use_mask:
                        mt = work.tile([128, 512], bf16, tag="msk", bufs=4,
                                       name="msk_t")
                        nc.sync.dma_start(out=mt,
                                          in_=MSK[ts(j, 128), ts(c, 512)])
                        msk_tiles[j] = mt
                    for hh in range(2):
                        hoff = hh * 64
                        ps = pp.tile([128, 512], f32, tag="sc", bufs=3,
                                     name="sc_ps")
                        nc.tensor.matmul(
                            ps[:, q0:512],
                            kt_sb[pr_i][hoff:hoff + 64, ts(j, 128)],
                            qt_sb[pr_i][hoff:hoff + 64, ds(c * 512 + q0, w)],
                            start=True, stop=True)
                        pr = probs_pool.tile([128, 512], bf16, tag="probs",
                                             bufs=36, name="probs_t")
                        nc.scalar.activation(out=pr[:, q0:512],
                                             in_=ps[:, q0:512],
                                             func=EXP, scale=SCALE)
                        if diag:
                            nc.vector.tensor_mul(pr[:, q0:q0 + 128],
                                                 pr[:, q0:q0 + 128], tri_sb)
                        if use_mask:
                            nc.vector.tensor_mul(pr[:, q0:512], pr[:, q0:512],
                                                 msk_tiles[j][:, q0:512])
                        probs[(j, hh)] = pr
                for hh in range(2):
                    h_loc = 2 * pr_i + hh
                    psA = pp.tile([D + 1, 512], f32, tag="attn", bufs=3,
                                  name="attn_ps")
                    for j in range(nj):
                        diag = causal and (j // 4 == c)
                        q0 = (j - 4 * c) * 128 if diag else 0
                        nc.tensor.matmul(psA[:, q0:512],
                                         v_sb[:, j, h_loc, :],
                                         probs[(j, hh)][:, q0:512],
                                         start=(j == 0), stop=(j == nj - 1))
                    recip = work.tile([1, 512], f32, tag="recip", bufs=4,
                                      name="recip_t")
                    nc.vector.reciprocal(recip, psA[D:D + 1, :])
                    bcast = work.tile([64, 512], f32, tag="bcast", bufs=4,
                                      name="bcast_t")
                    nc.gpsimd.partition_broadcast(bcast, recip)
                    nc.vector.tensor_mul(
                        at_sb[pr_i][hh * 64:hh * 64 + 64, ts(c, 512)],
                        psA[0:D, :], bcast)

        # ---- phase 3: output projection -------------------------------
        for tt in range(n_tt):
            for nch in range(E // 512):
                ps = pp.tile([128, 512], f32, tag="proj", bufs=2, name="out_ps")
                for kk in range(HD // 128):
                    nc.tensor.matmul(ps,
                                     at_sb[kk][:, ts(tt, 128)],
                                     wo_sb[kk][:, ts(nch, 512)],
                                     start=(kk == 0), stop=(kk == HD // 128 - 1))
                osb = work.tile([128, 512], f32, tag="osb", bufs=4,
                                name="osb_t")
                if (tt * 2 + nch) % 2 == 0:
                    nc.vector.tensor_copy(osb, ps)
                else:
                    nc.scalar.copy(osb, ps)
                nc.sync.dma_start(out=Y[ts(tt, 128), ts(nch, 512)], in_=osb)

    nc.compile()
    _BUILD_CACHE[key] = nc
    return nc


def make_in_maps(Q, K, V, Wq, bq, Wk, bk, Wv, bv, Wo, mask_mode, maskT=None,
                 seq_len=S):
    """Host-side shard + layout prep. Returns list of per-core input dicts."""
    tri = np.triu(np.ones((128, 128), dtype=np.float32)).astype(BF16)
    qkvT = []
    for b in range(B):
        qkvT.append((np.ascontiguousarray(Q[b].T).astype(BF16),
                     np.ascontiguousarray(K[b].T).astype(BF16),
                     np.ascontiguousarray(V[b].T).astype(BF16)))
    in_maps = []
    for c in range(N_CORES):
        b, g = c // GROUPS, c % GROUPS
        sl = slice(g * HD, (g + 1) * HD)
        qT, kT, vT = qkvT[b]
        m = {
            "qt_in": qT, "kt_in": kT, "vt_in": vT,
            "wqt": np.ascontiguousarray(Wq[sl, :].T).astype(BF16),
            "wkt": np.ascontiguousarray(Wk[sl, :].T).astype(BF16),
            "wvt": np.ascontiguousarray(Wv[sl, :].T).astype(BF16),
            "wot": np.ascontiguousarray(Wo[:, sl].T).astype(BF16),
            "bq_in": np.ascontiguousarray(bq[sl].reshape(HD, 1)).astype(np.float32),
            "bk_in": np.ascontiguousarray(bk[sl].reshape(HD, 1)).astype(np.float32),
            "bv_in": np.ascontiguousarray(bv[sl].reshape(1, HD)).astype(np.float32),
            "tri": tri,
        }
        if mask_mode == "generic":
            m["mskt"] = maskT
        in_maps.append(m)
    return in_maps


def _detect_mask_mode(mask):
    m = np.asarray(mask)
    m2 = m.reshape(m.shape[-2], m.shape[-1])
    if (m2 != 0).all():
        return "dense", None
    s = m2.shape[0]
    if np.array_equal(m2 != 0, np.tril(np.ones((s, s), dtype=bool))):
        return "causal", None
    return "generic", np.ascontiguousarray((m2 != 0).T.astype(BF16))


def kernel(Q, K, V, Wq, bq, Wk, bk, Wv, bv, Wo, bo, mask):
    from concourse.bass_utils import run_bass_kernel_spmd

    Q, K, V = (np.asarray(x, dtype=np.float32) for x in (Q, K, V))
    Wq, bq, Wk, bk, Wv, bv, Wo, bo = (
        np.asarray(x, dtype=np.float32)
        for x in (Wq, bq, Wk, bk, Wv, bv, Wo, bo))

    mode, maskT = _detect_mask_mode(mask)
    nc = build_nc(seq_len=S, causal=(mode == "causal"),
                  use_mask=(mode == "generic"))
    in_maps = make_in_maps(Q, K, V, Wq, bq, Wk, bk, Wv, bv, Wo,
                           mode, maskT)
    res = run_bass_kernel_spmd(nc, in_maps, list(range(N_CORES)))
    out = np.empty((B, S, E), dtype=np.float32)
    for b in range(B):
        acc = res.results[b * GROUPS]["y"].astype(np.float32).copy()
        for g in range(1, GROUPS):
            acc += res.results[b * GROUPS + g]["y"]
        out[b] = acc + bo[None, :]
    return out
